# revision 5
# baseline (speedup 1.0000x reference)
"""Bass/Trainium2 kernel for nn_Attention (additive attention + weighted sum).

Computation (reference):
    enc  = encoder_outputs.transpose(1, 0, 2)              # [B, S, E]
    z    = enc @ w_e.T + hidden @ w_h.T + attn_b           # [B, S, O]
    att  = softmax(tanh(z) @ v, axis=S)                    # [B, S]
    out  = att @ enc                                       # [B, E]

Sharding: data-parallel over batch — 8 cores x 4 batches each.
Host precomputes hidden @ w_h.T + attn_b (0.1% of FLOPs) and the two
encoder layouts each core streams: encT [b, e, s] for the big matmul
(contraction over e needs e on partitions) and encN [s, b, e] for the
attention-weighted sum (contraction over s).
"""

import numpy as np
from contextlib import ExitStack

# Problem shapes (hardcoded; kernel.py must be self-contained).
B = 32
S = 2048
E = 1024  # encoder hidden
O = 1024  # output dim / attention proj dim
N_CORES = 8
BL = B // N_CORES  # batches per core = 4

P = 128    # partitions
F = 512    # matmul moving free dim (one fp32 PSUM bank)
KE = E // P   # 8 contraction tiles over e
MT = O // P   # 8 output-row tiles over p
NCH = S // F  # 4 s-chunks in pass A
NT = S // P   # 16 s-tiles in pass B
ECH = E // F  # 2 e-chunks in pass B

_PROGRAM = None


def _build_program():
    import concourse.tile as tile
    from concourse import bacc, mybir

    f32 = mybir.dt.float32
    f32r = mybir.dt.float32r
    AF = mybir.ActivationFunctionType
    AX = mybir.AxisListType

    nc = bacc.Bacc("TRN2", target_bir_lowering=False, debug=False,
                   num_devices=N_CORES)

    encT = nc.dram_tensor("encT", [BL, E, S], f32r, kind="ExternalInput").ap()
    encN = nc.dram_tensor("encN", [S, BL, E], f32r, kind="ExternalInput").ap()
    weT = nc.dram_tensor("weT", [P, KE, O], f32r, kind="ExternalInput").ap()
    hb = nc.dram_tensor("hb", [P, MT, BL], f32, kind="ExternalInput").ap()
    vm = nc.dram_tensor("vm", [P, MT], f32r, kind="ExternalInput").ap()
    out = nc.dram_tensor("out", [BL, O], f32, kind="ExternalOutput").ap()

    with tile.TileContext(nc) as tc, ExitStack() as ctx:
        consts = ctx.enter_context(tc.tile_pool(name="consts", bufs=1))
        enc_pool = ctx.enter_context(tc.tile_pool(name="enc", bufs=24))
        encn_pool = ctx.enter_context(tc.tile_pool(name="encn", bufs=20))
        epool = ctx.enter_context(tc.tile_pool(name="energy", bufs=10))
        spool = ctx.enter_context(tc.tile_pool(name="scores", bufs=2))
        apool = ctx.enter_context(tc.tile_pool(name="asb", bufs=2))
        opool = ctx.enter_context(tc.tile_pool(name="outsb", bufs=2))
        small = ctx.enter_context(tc.tile_pool(name="small", bufs=4))
        psA = ctx.enter_context(tc.tile_pool(name="psA", bufs=3, space="PSUM"))
        psS = ctx.enter_context(tc.tile_pool(name="psS", bufs=2, space="PSUM"))
        psT = ctx.enter_context(tc.tile_pool(name="psT", bufs=1, space="PSUM"))
        psB = ctx.enter_context(tc.tile_pool(name="psB", bufs=2, space="PSUM"))

        # Resident constants. weT split per k-tile so the first matmuls
        # only wait on their own slice.
        weT_sb = consts.tile([P, KE, O], f32r)
        for k in range(KE):
            nc.sync.dma_start(weT_sb[:, k, :], weT[:, k, :])
        vm_sb = consts.tile([P, MT], f32r)
        nc.sync.dma_start(vm_sb[:], vm[:])
        hb_sb = consts.tile([P, MT, BL], f32)
        nc.sync.dma_start(hb_sb[:], hb[:])

        scores_rows = {}

        def pass_a(b):
            # scores[b, s] = v . tanh(w_e @ enc[s] + hb[b])  for all s
            srow = spool.tile([1, S], f32, tag="srow")
            scores_rows[b] = srow
            for c in range(NCH):
                ek = []
                for k in range(KE):
                    t = enc_pool.tile([P, F], f32r, tag="ek")
                    nc.sync.dma_start(
                        t[:], encT[b, k * P:(k + 1) * P, c * F:(c + 1) * F])
                    ek.append(t)
                energies = []
                for m in range(MT):
                    ps = psA.tile([P, F], f32, tag="psA")
                    for k in range(KE):
                        nc.tensor.matmul(
                            ps[:],
                            weT_sb[:, k, m * P:(m + 1) * P],
                            ek[k][:],
                            start=(k == 0), stop=(k == KE - 1))
                    energy = epool.tile([P, F], f32r, tag="energy")
                    nc.scalar.activation(energy[:], ps[:], AF.Tanh,
                                         bias=hb_sb[:, m, b:b + 1])
                    energies.append(energy)
                # v-dot after all main matmuls of this chunk: keeps PE from
                # stalling on the tanh of the tile it just produced.
                sps = psS.tile([1, F], f32, tag="sps")
                for m in range(MT):
                    nc.tensor.matmul(
                        sps[:], vm_sb[:, m:m + 1],
                        energies[m][:],
                        start=(m == 0), stop=(m == MT - 1))
                nc.scalar.copy(srow[:, c * F:(c + 1) * F], sps[:])

        def post(b):
            # softmax over s (length S) on a single-partition row, then
            # transpose the normalized weights to s-on-partitions via
            # K=1 matmuls, then weighted sum over s on PE.
            srow = scores_rows.pop(b)
            negmax = small.tile([1, 1], f32, tag="negmax")
            nc.vector.reduce_max(negmax[:], srow[:], axis=AX.X, negate=True)
            erow = spool.tile([1, S], f32, tag="erow")
            den = small.tile([1, 1], f32, tag="den")
            nc.scalar.activation(erow[:], srow[:], AF.Exp,
                                 bias=negmax[:], accum_out=den[:])
            rden = small.tile([1, 1], f32, tag="rden")
            nc.vector.reciprocal(rden[:], den[:])
            # aT[:, t] = erow[0, tP:(t+1)P] * (1/den): K=1 matmul puts the
            # row down the partitions and normalizes in the same shot.
            aps = psT.tile([P, NT], f32, tag="aT")
            for t in range(NT):
                nc.tensor.matmul(
                    aps[:, t:t + 1],
                    erow[:, t * P:(t + 1) * P],
                    rden[:],
                    start=True, stop=True)
            a_sb = apool.tile([P, NT], f32r, tag="a_sb")
            nc.vector.tensor_copy(a_sb[:], aps[:])

            osb = opool.tile([1, O], f32, tag="osb")
            for ec in range(ECH):
                bps = psB.tile([1, F], f32, tag="psB")
                for t in range(NT):
                    ent = encn_pool.tile([P, F], f32r, tag="ent")
                    nc.sync.dma_start(
                        ent[:],
                        encN[t * P:(t + 1) * P, b, ec * F:(ec + 1) * F])
                    nc.tensor.matmul(
                        bps[:], a_sb[:, t:t + 1],
                        ent[:],
                        start=(t == 0), stop=(t == NT - 1))
                nc.scalar.copy(osb[:, ec * F:(ec + 1) * F], bps[:])
            nc.sync.dma_start(out[b:b + 1, :], osb[:])

        # Software pipeline: post(b) overlaps pass_a(b+1) on the other engines.
        pass_a(0)
        pass_a(1)
        post(0)
        pass_a(2)
        post(1)
        pass_a(3)
        post(2)
        post(3)

    nc.compile()
    return nc


def _get_program():
    global _PROGRAM
    if _PROGRAM is None:
        _PROGRAM = _build_program()
    return _PROGRAM


def _make_in_maps(hidden, encoder_outputs, attn_w, attn_b, v):
    hidden = np.asarray(hidden, dtype=np.float32)
    enc = np.asarray(encoder_outputs, dtype=np.float32)
    attn_w = np.asarray(attn_w, dtype=np.float32)
    attn_b = np.asarray(attn_b, dtype=np.float32)
    v = np.asarray(v, dtype=np.float32)

    hb_full = hidden @ attn_w[:, :O].T + attn_b          # [B, O]
    weT = np.ascontiguousarray(
        attn_w[:, O:].T.reshape(KE, P, O).transpose(1, 0, 2))  # [P, KE, O]
    vm = np.ascontiguousarray(v.reshape(MT, P).T)        # [P, MT]

    in_maps = []
    for core in range(N_CORES):
        sl = slice(core * BL, (core + 1) * BL)
        encN_c = np.ascontiguousarray(enc[:, sl, :])     # [S, BL, E]
        encT_c = np.ascontiguousarray(encN_c.transpose(1, 2, 0))  # [BL, E, S]
        hb_c = np.ascontiguousarray(
            hb_full[sl].T.reshape(MT, P, BL).transpose(1, 0, 2))  # [P, MT, BL]
        in_maps.append({
            "encT": encT_c,
            "encN": encN_c,
            "weT": weT,
            "hb": hb_c,
            "vm": vm,
        })
    return in_maps


def run(trace=False, **inputs):
    from concourse.bass_utils import run_bass_kernel_spmd
    nc = _get_program()
    in_maps = _make_in_maps(**inputs)
    res = run_bass_kernel_spmd(nc, in_maps, list(range(N_CORES)), trace=trace)
    outp = np.concatenate([res.results[i]["out"] for i in range(N_CORES)],
                          axis=0)
    return outp, res


def kernel(**inputs) -> np.ndarray:
    outp, _ = run(trace=False, **inputs)
    return outp


# revision 8
# speedup vs baseline: 1.0162x; 1.0162x over previous
"""Bass/Trainium2 kernel for nn_Attention (additive attention + weighted sum).

Computation (reference):
    enc  = encoder_outputs.transpose(1, 0, 2)              # [B, S, E]
    z    = enc @ w_e.T + hidden @ w_h.T + attn_b           # [B, S, O]
    att  = softmax(tanh(z) @ v, axis=S)                    # [B, S]
    out  = att @ enc                                       # [B, E]

Sharding: data-parallel over batch — 8 cores x 4 batches each.
Host precomputes hidden @ w_h.T + attn_b (0.1% of FLOPs) and the two
encoder layouts each core streams: encT [b, e, s] for the big matmul
(contraction over e needs e on partitions) and encN [s, b, e] for the
attention-weighted sum (contraction over s).
"""

import numpy as np
from contextlib import ExitStack

# Problem shapes (hardcoded; kernel.py must be self-contained).
B = 32
S = 2048
E = 1024  # encoder hidden
O = 1024  # output dim / attention proj dim
N_CORES = 8
BL = B // N_CORES  # batches per core = 4

P = 128    # partitions
F = 512    # matmul moving free dim (one fp32 PSUM bank)
KE = E // P   # 8 contraction tiles over e
MT = O // P   # 8 output-row tiles over p
NCH = S // F  # 4 s-chunks in pass A
NT = S // P   # 16 s-tiles in pass B
ECH = E // F  # 2 e-chunks in pass B

_PROGRAM = None


def _build_program():
    import concourse.tile as tile
    from concourse import bacc, mybir

    f32 = mybir.dt.float32
    f32r = mybir.dt.float32r
    AF = mybir.ActivationFunctionType
    AX = mybir.AxisListType

    nc = bacc.Bacc("TRN2", target_bir_lowering=False, debug=False,
                   num_devices=N_CORES)

    encT = nc.dram_tensor("encT", [BL, E, S], f32r, kind="ExternalInput").ap()
    encN = nc.dram_tensor("encN", [S, BL, E], f32r, kind="ExternalInput").ap()
    weT = nc.dram_tensor("weT", [P, KE, O], f32r, kind="ExternalInput").ap()
    hb = nc.dram_tensor("hb", [P, MT, BL], f32, kind="ExternalInput").ap()
    vm = nc.dram_tensor("vm", [P, MT], f32r, kind="ExternalInput").ap()
    out = nc.dram_tensor("out", [BL, O], f32, kind="ExternalOutput").ap()

    with tile.TileContext(nc) as tc, ExitStack() as ctx:
        consts = ctx.enter_context(tc.tile_pool(name="consts", bufs=1))
        enc_pool = ctx.enter_context(tc.tile_pool(name="enc", bufs=20))
        encn_pool = ctx.enter_context(tc.tile_pool(name="encn", bufs=10))
        epool = ctx.enter_context(tc.tile_pool(name="energy", bufs=10))
        spool = ctx.enter_context(tc.tile_pool(name="scores", bufs=2))
        apool = ctx.enter_context(tc.tile_pool(name="asb", bufs=2))
        opool = ctx.enter_context(tc.tile_pool(name="outsb", bufs=2))
        small = ctx.enter_context(tc.tile_pool(name="small", bufs=4))
        # One shared PSUM pool: every tile uses the same tag so any mix of
        # up to 8 concurrently-live psum tiles (8 banks) is legal.
        pps = ctx.enter_context(tc.tile_pool(name="pps", bufs=8, space="PSUM"))

        def ps_tile():
            return pps.tile([P, F], f32, tag="ps", name="ps")

        weT_sb = consts.tile([P, KE, O], f32r)
        vm_sb = consts.tile([P, MT], f32r)
        hb_sb = consts.tile([P, MT, BL], f32)

        scores_rows = {}

        def load_ek(b, c, k):
            t = enc_pool.tile([P, F], f32r, tag="ek")
            nc.sync.dma_start(
                t[:], encT[b, k * P:(k + 1) * P, c * F:(c + 1) * F])
            return t

        def chunk_tail(b, c, srow, energies):
            # v-dot after all main matmuls of this chunk: keeps PE from
            # stalling on the tanh of the tile it just produced.
            sps = ps_tile()
            for m in range(MT):
                nc.tensor.matmul(
                    sps[:1, :], vm_sb[:, m:m + 1], energies[m][:],
                    start=(m == 0), stop=(m == MT - 1))
            nc.scalar.copy(srow[:, c * F:(c + 1) * F], sps[:1, :])

        def pass_a_first(b):
            # Startup chunk (b0,c0): k-blocked so the first matmuls only
            # need weT[k]+ek[k] (0.75 MB) instead of the full 6 MB
            # prefetch. Uses 8 psum banks (one per m-tile).
            srow = spool.tile([1, S], f32, tag="srow")
            scores_rows[b] = srow
            ek = []
            for k in range(KE):
                # interleave: weights slice k, then data tile k
                nc.sync.dma_start(weT_sb[:, k, :], weT[:, k, :])
                ek.append(load_ek(b, 0, k))
            nc.sync.dma_start(vm_sb[:], vm[:])
            nc.sync.dma_start(hb_sb[:], hb[:])
            pstiles = [ps_tile() for _ in range(MT)]
            for k in range(KE):
                for m in range(MT):
                    nc.tensor.matmul(
                        pstiles[m][:], weT_sb[:, k, m * P:(m + 1) * P],
                        ek[k][:], start=(k == 0), stop=(k == KE - 1))
            energies = []
            for m in range(MT):
                energy = epool.tile([P, F], f32r, tag="energy")
                nc.scalar.activation(energy[:], pstiles[m][:], AF.Tanh,
                                     bias=hb_sb[:, m, b:b + 1])
                energies.append(energy)
            chunk_tail(b, 0, srow, energies)
            # remaining chunks of b0 use the steady-state order
            for c in range(1, NCH):
                pass_a_chunk(b, c, srow)

        def pass_a_chunk(b, c, srow):
            ek = [load_ek(b, c, k) for k in range(KE)]
            energies = []
            for m in range(MT):
                ps = ps_tile()
                for k in range(KE):
                    nc.tensor.matmul(
                        ps[:], weT_sb[:, k, m * P:(m + 1) * P], ek[k][:],
                        start=(k == 0), stop=(k == KE - 1))
                energy = epool.tile([P, F], f32r, tag="energy")
                nc.scalar.activation(energy[:], ps[:], AF.Tanh,
                                     bias=hb_sb[:, m, b:b + 1])
                energies.append(energy)
            chunk_tail(b, c, srow, energies)

        def pass_a(b):
            srow = spool.tile([1, S], f32, tag="srow")
            scores_rows[b] = srow
            for c in range(NCH):
                pass_a_chunk(b, c, srow)

        def post(b):
            # softmax over s (length S) on a single-partition row, then
            # transpose the normalized weights to s-on-partitions via
            # K=1 matmuls, then the weighted sum over s on PE.
            srow = scores_rows.pop(b)
            # prefetch the whole encN[b] slab first: 16 tiles x [128,1024]
            # (4 KB contiguous per partition), consumed t-outer/ec-inner.
            ents = []
            for t in range(NT):
                ent = encn_pool.tile([P, E], f32r, tag="ent")
                nc.sync.dma_start(ent[:], encN[t * P:(t + 1) * P, b, :])
                ents.append(ent)
            negmax = small.tile([1, 1], f32, tag="negmax")
            nc.vector.reduce_max(negmax[:], srow[:], axis=AX.X, negate=True)
            erow = spool.tile([1, S], f32, tag="erow")
            den = small.tile([1, 1], f32, tag="den")
            nc.scalar.activation(erow[:], srow[:], AF.Exp,
                                 bias=negmax[:], accum_out=den[:])
            rden = small.tile([1, 1], f32, tag="rden")
            nc.vector.reciprocal(rden[:], den[:])
            # aT[:, t] = erow[0, tP:(t+1)P] * (1/den): K=1 matmul puts the
            # row down the partitions and normalizes in the same shot.
            aps = pps.tile([P, F], f32, tag="ps", name="aps")
            for t in range(NT):
                nc.tensor.matmul(
                    aps[:, t:t + 1],
                    erow[:, t * P:(t + 1) * P],
                    rden[:],
                    start=True, stop=True)
            a_sb = apool.tile([P, NT], f32r, tag="a_sb")
            nc.vector.tensor_copy(a_sb[:], aps[:, :NT])

            osb = opool.tile([1, O], f32, tag="osb")
            bps = [ps_tile() for _ in range(ECH)]
            for ec in range(ECH):
                for t in range(NT):
                    nc.tensor.matmul(
                        bps[ec][:1, :], a_sb[:, t:t + 1],
                        ents[t][:, ec * F:(ec + 1) * F],
                        start=(t == 0), stop=(t == NT - 1))
            for ec in range(ECH):
                nc.scalar.copy(osb[:, ec * F:(ec + 1) * F], bps[ec][:1, :])
            nc.sync.dma_start(out[b:b + 1, :], osb[:])

        # Software pipeline: post(b) overlaps pass_a(b+1) on the other engines.
        pass_a_first(0)
        pass_a(1)
        post(0)
        pass_a(2)
        post(1)
        pass_a(3)
        post(2)
        post(3)

    nc.compile()
    return nc


def _get_program():
    global _PROGRAM
    if _PROGRAM is None:
        _PROGRAM = _build_program()
    return _PROGRAM


def _make_in_maps(hidden, encoder_outputs, attn_w, attn_b, v):
    hidden = np.asarray(hidden, dtype=np.float32)
    enc = np.asarray(encoder_outputs, dtype=np.float32)
    attn_w = np.asarray(attn_w, dtype=np.float32)
    attn_b = np.asarray(attn_b, dtype=np.float32)
    v = np.asarray(v, dtype=np.float32)

    hb_full = hidden @ attn_w[:, :O].T + attn_b          # [B, O]
    weT = np.ascontiguousarray(
        attn_w[:, O:].T.reshape(KE, P, O).transpose(1, 0, 2))  # [P, KE, O]
    vm = np.ascontiguousarray(v.reshape(MT, P).T)        # [P, MT]

    in_maps = []
    for core in range(N_CORES):
        sl = slice(core * BL, (core + 1) * BL)
        encN_c = np.ascontiguousarray(enc[:, sl, :])     # [S, BL, E]
        encT_c = np.ascontiguousarray(encN_c.transpose(1, 2, 0))  # [BL, E, S]
        hb_c = np.ascontiguousarray(
            hb_full[sl].T.reshape(MT, P, BL).transpose(1, 0, 2))  # [P, MT, BL]
        in_maps.append({
            "encT": encT_c,
            "encN": encN_c,
            "weT": weT,
            "hb": hb_c,
            "vm": vm,
        })
    return in_maps


def run(trace=False, **inputs):
    from concourse.bass_utils import run_bass_kernel_spmd
    nc = _get_program()
    in_maps = _make_in_maps(**inputs)
    res = run_bass_kernel_spmd(nc, in_maps, list(range(N_CORES)), trace=trace)
    outp = np.concatenate([res.results[i]["out"] for i in range(N_CORES)],
                          axis=0)
    return outp, res


def kernel(**inputs) -> np.ndarray:
    outp, _ = run(trace=False, **inputs)
    return outp


# revision 9
# speedup vs baseline: 1.0385x; 1.0220x over previous
"""Bass/Trainium2 kernel for nn_Attention (additive attention + weighted sum).

Computation (reference):
    enc  = encoder_outputs.transpose(1, 0, 2)              # [B, S, E]
    z    = enc @ w_e.T + hidden @ w_h.T + attn_b           # [B, S, O]
    att  = softmax(tanh(z) @ v, axis=S)                    # [B, S]
    out  = att @ enc                                       # [B, E]

Sharding: data-parallel over batch — 8 cores x 4 batches each.
Host precomputes hidden @ w_h.T + attn_b (0.1% of FLOPs) and the two
encoder layouts each core streams: encT [b, e, s] for the big matmul
(contraction over e needs e on partitions) and encN [s, b, e] for the
attention-weighted sum (contraction over s).
"""

import numpy as np
from contextlib import ExitStack

# Problem shapes (hardcoded; kernel.py must be self-contained).
B = 32
S = 2048
E = 1024  # encoder hidden
O = 1024  # output dim / attention proj dim
N_CORES = 8
BL = B // N_CORES  # batches per core = 4

P = 128    # partitions
F = 512    # matmul moving free dim (one fp32 PSUM bank)
KE = E // P   # 8 contraction tiles over e
MT = O // P   # 8 output-row tiles over p
NCH = S // F  # 4 s-chunks in pass A
NT = S // P   # 16 s-tiles in pass B
ECH = E // F  # 2 e-chunks in pass B

_PROGRAM = None


def _build_program():
    import concourse.tile as tile
    from concourse import bacc, mybir

    f32 = mybir.dt.float32
    f32r = mybir.dt.float32r
    AF = mybir.ActivationFunctionType
    AX = mybir.AxisListType

    nc = bacc.Bacc("TRN2", target_bir_lowering=False, debug=False,
                   num_devices=N_CORES)

    encT = nc.dram_tensor("encT", [BL, E, S], f32r, kind="ExternalInput").ap()
    encN = nc.dram_tensor("encN", [S, BL, E], f32r, kind="ExternalInput").ap()
    weT = nc.dram_tensor("weT", [P, KE, O], f32r, kind="ExternalInput").ap()
    hb = nc.dram_tensor("hb", [P, MT, BL], f32, kind="ExternalInput").ap()
    vm = nc.dram_tensor("vm", [P, MT], f32r, kind="ExternalInput").ap()
    out = nc.dram_tensor("out", [BL, O], f32, kind="ExternalOutput").ap()

    with tile.TileContext(nc) as tc, ExitStack() as ctx:
        consts = ctx.enter_context(tc.tile_pool(name="consts", bufs=1))
        enc_pool = ctx.enter_context(tc.tile_pool(name="enc", bufs=16))
        encn_pool = ctx.enter_context(tc.tile_pool(name="encn", bufs=17))
        epool = ctx.enter_context(tc.tile_pool(name="energy", bufs=8))
        spool = ctx.enter_context(tc.tile_pool(name="scores", bufs=2))
        apool = ctx.enter_context(tc.tile_pool(name="asb", bufs=2))
        opool = ctx.enter_context(tc.tile_pool(name="outsb", bufs=2))
        small = ctx.enter_context(tc.tile_pool(name="small", bufs=4))
        # One shared PSUM pool: every tile uses the same tag so any mix of
        # up to 8 concurrently-live psum tiles (8 banks) is legal.
        pps = ctx.enter_context(tc.tile_pool(name="pps", bufs=8, space="PSUM"))

        def ps_tile():
            return pps.tile([P, F], f32, tag="ps", name="ps")

        weT_sb = consts.tile([P, KE, O], f32r)
        vm_sb = consts.tile([P, MT], f32r)
        hb_sb = consts.tile([P, MT, BL], f32)

        scores_rows = {}

        def load_ek(b, c, k):
            t = enc_pool.tile([P, F], f32r, tag="ek")
            nc.sync.dma_start(
                t[:], encT[b, k * P:(k + 1) * P, c * F:(c + 1) * F])
            return t

        def chunk_tail(b, c, srow, energies):
            # v-dot after all main matmuls of this chunk: keeps PE from
            # stalling on the tanh of the tile it just produced.
            sps = ps_tile()
            for m in range(MT):
                nc.tensor.matmul(
                    sps[:1, :], vm_sb[:, m:m + 1], energies[m][:],
                    start=(m == 0), stop=(m == MT - 1))
            nc.scalar.copy(srow[:, c * F:(c + 1) * F], sps[:1, :])

        def pass_a_first(b):
            # Startup chunk (b0,c0): k-blocked so the first matmuls only
            # need weT[k]+ek[k] (0.75 MB) instead of the full 6 MB
            # prefetch. Uses 8 psum banks (one per m-tile).
            srow = spool.tile([1, S], f32, tag="srow")
            scores_rows[b] = srow
            ek = []
            for k in range(KE):
                # interleave: weights slice k, then data tile k; slice k=0
                # per m-tile so the very first matmul waits on <1 MB.
                if k == 0:
                    for m in range(MT):
                        nc.sync.dma_start(weT_sb[:, 0, m * P:(m + 1) * P],
                                          weT[:, 0, m * P:(m + 1) * P])
                else:
                    nc.sync.dma_start(weT_sb[:, k, :], weT[:, k, :])
                ek.append(load_ek(b, 0, k))
            nc.sync.dma_start(vm_sb[:], vm[:])
            nc.sync.dma_start(hb_sb[:], hb[:])
            pstiles = [ps_tile() for _ in range(MT)]
            for k in range(KE):
                for m in range(MT):
                    nc.tensor.matmul(
                        pstiles[m][:], weT_sb[:, k, m * P:(m + 1) * P],
                        ek[k][:], start=(k == 0), stop=(k == KE - 1))
            energies = []
            for m in range(MT):
                energy = epool.tile([P, F], f32r, tag="energy")
                nc.scalar.activation(energy[:], pstiles[m][:], AF.Tanh,
                                     bias=hb_sb[:, m, b:b + 1])
                energies.append(energy)
            chunk_tail(b, 0, srow, energies)
            # remaining chunks of b0 use the steady-state order
            for c in range(1, NCH):
                pass_a_chunk(b, c, srow)

        def pass_a_chunk(b, c, srow):
            ek = [load_ek(b, c, k) for k in range(KE)]
            energies = []
            for m in range(MT):
                ps = ps_tile()
                for k in range(KE):
                    nc.tensor.matmul(
                        ps[:], weT_sb[:, k, m * P:(m + 1) * P], ek[k][:],
                        start=(k == 0), stop=(k == KE - 1))
                energy = epool.tile([P, F], f32r, tag="energy")
                nc.scalar.activation(energy[:], ps[:], AF.Tanh,
                                     bias=hb_sb[:, m, b:b + 1])
                energies.append(energy)
            chunk_tail(b, c, srow, energies)

        def pass_a(b):
            srow = spool.tile([1, S], f32, tag="srow")
            scores_rows[b] = srow
            for c in range(NCH):
                pass_a_chunk(b, c, srow)

        ents_map = {}

        def post_dma(b):
            # prefetch the whole encN[b] slab: 16 tiles x [128,1024]
            # (4 KB contiguous per partition), consumed by post_compute(b).
            ents = []
            for t in range(NT):
                ent = encn_pool.tile([P, E], f32r, tag="ent")
                nc.sync.dma_start(ent[:], encN[t * P:(t + 1) * P, b, :])
                ents.append(ent)
            ents_map[b] = ents

        def post_compute(b):
            # softmax over s (length S) on a single-partition row, then
            # transpose the normalized weights to s-on-partitions via
            # K=1 matmuls, then the weighted sum over s on PE.
            srow = scores_rows.pop(b)
            ents = ents_map.pop(b)
            negmax = small.tile([1, 1], f32, tag="negmax")
            nc.vector.reduce_max(negmax[:], srow[:], axis=AX.X, negate=True)
            erow = spool.tile([1, S], f32, tag="erow")
            den = small.tile([1, 1], f32, tag="den")
            nc.scalar.activation(erow[:], srow[:], AF.Exp,
                                 bias=negmax[:], accum_out=den[:])
            rden = small.tile([1, 1], f32, tag="rden")
            nc.vector.reciprocal(rden[:], den[:])
            # aT[:, t] = erow[0, tP:(t+1)P] * (1/den): K=1 matmul puts the
            # row down the partitions and normalizes in the same shot.
            aps = pps.tile([P, F], f32, tag="ps", name="aps")
            for t in range(NT):
                nc.tensor.matmul(
                    aps[:, t:t + 1],
                    erow[:, t * P:(t + 1) * P],
                    rden[:],
                    start=True, stop=True)
            a_sb = apool.tile([P, NT], f32r, tag="a_sb")
            nc.vector.tensor_copy(a_sb[:], aps[:, :NT])

            osb = opool.tile([1, O], f32, tag="osb")
            bps = [ps_tile() for _ in range(ECH)]
            for ec in range(ECH):
                for t in range(NT):
                    nc.tensor.matmul(
                        bps[ec][:1, :], a_sb[:, t:t + 1],
                        ents[t][:, ec * F:(ec + 1) * F],
                        start=(t == 0), stop=(t == NT - 1))
            for ec in range(ECH):
                nc.scalar.copy(osb[:, ec * F:(ec + 1) * F], bps[ec][:1, :])
            nc.sync.dma_start(out[b:b + 1, :], osb[:])

        # Software pipeline: post_compute(b) overlaps pass_a(b+1) on the
        # other engines; post_dma(b) rides the DMA queue right behind
        # pass_a(b)'s own tiles so the tail is never DMA-starved.
        pass_a_first(0)
        post_dma(0)
        pass_a(1)
        post_compute(0)
        post_dma(1)
        pass_a(2)
        post_compute(1)
        post_dma(2)
        pass_a(3)
        post_compute(2)
        post_dma(3)
        post_compute(3)

    nc.compile()
    return nc


def _get_program():
    global _PROGRAM
    if _PROGRAM is None:
        _PROGRAM = _build_program()
    return _PROGRAM


def _make_in_maps(hidden, encoder_outputs, attn_w, attn_b, v):
    hidden = np.asarray(hidden, dtype=np.float32)
    enc = np.asarray(encoder_outputs, dtype=np.float32)
    attn_w = np.asarray(attn_w, dtype=np.float32)
    attn_b = np.asarray(attn_b, dtype=np.float32)
    v = np.asarray(v, dtype=np.float32)

    hb_full = hidden @ attn_w[:, :O].T + attn_b          # [B, O]
    weT = np.ascontiguousarray(
        attn_w[:, O:].T.reshape(KE, P, O).transpose(1, 0, 2))  # [P, KE, O]
    vm = np.ascontiguousarray(v.reshape(MT, P).T)        # [P, MT]

    in_maps = []
    for core in range(N_CORES):
        sl = slice(core * BL, (core + 1) * BL)
        encN_c = np.ascontiguousarray(enc[:, sl, :])     # [S, BL, E]
        encT_c = np.ascontiguousarray(encN_c.transpose(1, 2, 0))  # [BL, E, S]
        hb_c = np.ascontiguousarray(
            hb_full[sl].T.reshape(MT, P, BL).transpose(1, 0, 2))  # [P, MT, BL]
        in_maps.append({
            "encT": encT_c,
            "encN": encN_c,
            "weT": weT,
            "hb": hb_c,
            "vm": vm,
        })
    return in_maps


def run(trace=False, **inputs):
    from concourse.bass_utils import run_bass_kernel_spmd
    nc = _get_program()
    in_maps = _make_in_maps(**inputs)
    res = run_bass_kernel_spmd(nc, in_maps, list(range(N_CORES)), trace=trace)
    outp = np.concatenate([res.results[i]["out"] for i in range(N_CORES)],
                          axis=0)
    return outp, res


def kernel(**inputs) -> np.ndarray:
    outp, _ = run(trace=False, **inputs)
    return outp


# revision 11
# speedup vs baseline: 1.1038x; 1.0629x over previous
"""Bass/Trainium2 kernel for nn_Attention (additive attention + weighted sum).

Computation (reference):
    enc  = encoder_outputs.transpose(1, 0, 2)              # [B, S, E]
    z    = enc @ w_e.T + hidden @ w_h.T + attn_b           # [B, S, O]
    att  = softmax(tanh(z) @ v, axis=S)                    # [B, S]
    out  = att @ enc                                       # [B, E]

Sharding: data-parallel over batch — 8 cores x 4 batches each.
Host precomputes hidden @ w_h.T + attn_b (0.1% of FLOPs) and ships the
encoder slice in [b, e, s] layout (contraction over e needs e on
partitions for the big matmul).

Per core: pass A streams encT tiles through the PE (energy = tanh(
w_e @ enc + bias), then scores = v . energy as M=1 matmuls). The
attention-weighted sum reuses the SAME encT tiles on the vector engine
(tensor_tensor_reduce over the free/s axis with the softmaxed row
partition-broadcast), so the encoder is read from HBM exactly once and
the PE does nothing but the two contractions.
"""

import numpy as np
from contextlib import ExitStack

# Problem shapes (hardcoded; kernel.py must be self-contained).
B = 32
S = 2048
E = 1024  # encoder hidden
O = 1024  # output dim / attention proj dim
N_CORES = 8
BL = B // N_CORES  # batches per core = 4

P = 128    # partitions
F = 512    # matmul moving free dim (one fp32 PSUM bank)
KE = E // P   # 8 contraction tiles over e
MT = O // P   # 8 output-row tiles over p
NCH = S // F  # 4 s-chunks in pass A

_PROGRAM = None


def _build_program():
    import concourse.tile as tile
    from concourse import bacc, mybir

    f32 = mybir.dt.float32
    f32r = mybir.dt.float32r
    AF = mybir.ActivationFunctionType
    AX = mybir.AxisListType

    nc = bacc.Bacc("TRN2", target_bir_lowering=False, debug=False,
                   num_devices=N_CORES)

    encT = nc.dram_tensor("encT", [BL, E, S], f32r, kind="ExternalInput").ap()
    weT = nc.dram_tensor("weT", [P, KE, O], f32r, kind="ExternalInput").ap()
    hb = nc.dram_tensor("hb", [P, MT, BL], f32, kind="ExternalInput").ap()
    vm = nc.dram_tensor("vm", [P, MT], f32r, kind="ExternalInput").ap()
    # out[b, ep, kt] = weighted[b, kt*128 + ep]; host transposes back.
    out = nc.dram_tensor("out", [BL, P, KE], f32, kind="ExternalOutput").ap()

    with tile.TileContext(nc) as tc, ExitStack() as ctx:
        consts = ctx.enter_context(tc.tile_pool(name="consts", bufs=1))
        # encT tiles live from their pass-A matmuls until the vector-engine
        # weighted sum in post(b): one full batch (32 tiles) plus the next
        # batch streaming in.
        enc_pool = ctx.enter_context(tc.tile_pool(name="enc", bufs=48))
        epool = ctx.enter_context(tc.tile_pool(name="energy", bufs=8))
        spool = ctx.enter_context(tc.tile_pool(name="scores", bufs=2))
        bpool = ctx.enter_context(tc.tile_pool(name="bcast", bufs=2))
        jpool = ctx.enter_context(tc.tile_pool(name="junk", bufs=2))
        acpool = ctx.enter_context(tc.tile_pool(name="acc", bufs=2))
        opool = ctx.enter_context(tc.tile_pool(name="outsb", bufs=2))
        small = ctx.enter_context(tc.tile_pool(name="small", bufs=6))
        pps = ctx.enter_context(tc.tile_pool(name="pps", bufs=8, space="PSUM"))

        def ps_tile():
            return pps.tile([P, F], f32, tag="ps", name="ps")

        weT_sb = consts.tile([P, KE, O], f32r)
        vm_sb = consts.tile([P, MT], f32r)
        hb_sb = consts.tile([P, MT, BL], f32)

        scores_rows = {}
        ek_map = {}

        def load_ek(b, c, k):
            t = enc_pool.tile([P, F], f32r, tag="ek")
            nc.sync.dma_start(
                t[:], encT[b, k * P:(k + 1) * P, c * F:(c + 1) * F])
            return t

        def chunk_tail(b, c, srow, energies):
            # v-dot after all main matmuls of this chunk: keeps PE from
            # stalling on the tanh of the tile it just produced.
            sps = ps_tile()
            for m in range(MT):
                nc.tensor.matmul(
                    sps[:1, :], vm_sb[:, m:m + 1], energies[m][:],
                    start=(m == 0), stop=(m == MT - 1))
            nc.scalar.copy(srow[:, c * F:(c + 1) * F], sps[:1, :])

        def pass_a_first(b):
            # Startup chunk (b0,c0): k-blocked so the first matmuls only
            # need weT[k]+ek[k] (0.75 MB) instead of the full 6 MB
            # prefetch. Uses 8 psum banks (one per m-tile).
            srow = spool.tile([1, S], f32, tag="srow")
            scores_rows[b] = srow
            ek = []
            for k in range(KE):
                # interleave: weights slice k, then data tile k; slice k=0
                # per m-tile so the very first matmul waits on <1 MB.
                if k == 0:
                    for m in range(MT):
                        nc.sync.dma_start(weT_sb[:, 0, m * P:(m + 1) * P],
                                          weT[:, 0, m * P:(m + 1) * P])
                else:
                    nc.sync.dma_start(weT_sb[:, k, :], weT[:, k, :])
                ek.append(load_ek(b, 0, k))
            ek_map[(b, 0)] = ek
            nc.sync.dma_start(vm_sb[:], vm[:])
            nc.sync.dma_start(hb_sb[:], hb[:])
            pstiles = [ps_tile() for _ in range(MT)]
            for k in range(KE):
                for m in range(MT):
                    nc.tensor.matmul(
                        pstiles[m][:], weT_sb[:, k, m * P:(m + 1) * P],
                        ek[k][:], start=(k == 0), stop=(k == KE - 1))
            energies = []
            for m in range(MT):
                energy = epool.tile([P, F], f32r, tag="energy")
                nc.scalar.activation(energy[:], pstiles[m][:], AF.Tanh,
                                     bias=hb_sb[:, m, b:b + 1])
                energies.append(energy)
            chunk_tail(b, 0, srow, energies)
            for c in range(1, NCH):
                pass_a_chunk(b, c, srow)

        def pass_a_chunk(b, c, srow):
            ek = [load_ek(b, c, k) for k in range(KE)]
            ek_map[(b, c)] = ek
            energies = []
            for m in range(MT):
                ps = ps_tile()
                for k in range(KE):
                    nc.tensor.matmul(
                        ps[:], weT_sb[:, k, m * P:(m + 1) * P], ek[k][:],
                        start=(k == 0), stop=(k == KE - 1))
                energy = epool.tile([P, F], f32r, tag="energy")
                nc.scalar.activation(energy[:], ps[:], AF.Tanh,
                                     bias=hb_sb[:, m, b:b + 1])
                energies.append(energy)
            chunk_tail(b, c, srow, energies)

        def pass_a(b):
            srow = spool.tile([1, S], f32, tag="srow")
            scores_rows[b] = srow
            for c in range(NCH):
                pass_a_chunk(b, c, srow)

        def post(b):
            # Softmax over s on the single-partition score row, broadcast
            # the unnormalized weights across partitions, then the weighted
            # sum over s happens on the vector engine directly from the
            # resident encT tiles (free-axis multiply+reduce); 1/den is
            # applied per-partition on the final [P, KE] accumulator.
            srow = scores_rows.pop(b)
            negmax = small.tile([1, 1], f32, tag="negmax")
            nc.vector.reduce_max(negmax[:], srow[:], axis=AX.X, negate=True)
            erow = spool.tile([1, S], f32, tag="erow")
            den = small.tile([1, 1], f32, tag="den")
            nc.scalar.activation(erow[:], srow[:], AF.Exp,
                                 bias=negmax[:], accum_out=den[:])
            rden = small.tile([1, 1], f32, tag="rden")
            nc.vector.reciprocal(rden[:], den[:])
            erow_bc = bpool.tile([P, S], f32, tag="erow_bc")
            nc.gpsimd.partition_broadcast(erow_bc[:], erow[:])
            rden_bc = bpool.tile([P, 1], f32, tag="rden_bc")
            nc.gpsimd.partition_broadcast(rden_bc[:], rden[:])

            acc = acpool.tile([P, KE, NCH], f32, tag="acc")
            for c in range(NCH):
                ek = ek_map.pop((b, c))
                for k in range(KE):
                    prod = jpool.tile([P, F], f32, tag="junk", name="prod")
                    nc.vector.tensor_tensor(
                        prod[:], ek[k][:].bitcast(f32),
                        erow_bc[:, c * F:(c + 1) * F], mybir.AluOpType.mult)
                    nc.vector.reduce_sum(acc[:, k, c:c + 1], prod[:],
                                         axis=AX.X)
            accf = acpool.tile([P, KE], f32, tag="accf")
            nc.vector.reduce_sum(accf[:], acc[:], axis=AX.X)
            osb = opool.tile([P, KE], f32, tag="osb")
            nc.scalar.activation(osb[:], accf[:], AF.Copy,
                                 scale=rden_bc[:])
            nc.sync.dma_start(out[b], osb[:])

        # post(b) has no PE instructions, so the PE streams straight from
        # pass_a(b) into pass_a(b+1); post(b)'s DVE/ACT/GpSimd work hides
        # under pass_a(b+1)'s matmuls.
        pass_a_first(0)
        post(0)
        pass_a(1)
        post(1)
        pass_a(2)
        post(2)
        pass_a(3)
        post(3)

    nc.compile()
    return nc


def _get_program():
    global _PROGRAM
    if _PROGRAM is None:
        _PROGRAM = _build_program()
    return _PROGRAM


def _make_in_maps(hidden, encoder_outputs, attn_w, attn_b, v):
    hidden = np.asarray(hidden, dtype=np.float32)
    enc = np.asarray(encoder_outputs, dtype=np.float32)
    attn_w = np.asarray(attn_w, dtype=np.float32)
    attn_b = np.asarray(attn_b, dtype=np.float32)
    v = np.asarray(v, dtype=np.float32)

    hb_full = hidden @ attn_w[:, :O].T + attn_b          # [B, O]
    weT = np.ascontiguousarray(
        attn_w[:, O:].T.reshape(KE, P, O).transpose(1, 0, 2))  # [P, KE, O]
    vm = np.ascontiguousarray(v.reshape(MT, P).T)        # [P, MT]

    in_maps = []
    for core in range(N_CORES):
        sl = slice(core * BL, (core + 1) * BL)
        encT_c = np.ascontiguousarray(
            enc[:, sl, :].transpose(1, 2, 0))            # [BL, E, S]
        hb_c = np.ascontiguousarray(
            hb_full[sl].T.reshape(MT, P, BL).transpose(1, 0, 2))  # [P, MT, BL]
        in_maps.append({
            "encT": encT_c,
            "weT": weT,
            "hb": hb_c,
            "vm": vm,
        })
    return in_maps


def run(trace=False, **inputs):
    from concourse.bass_utils import run_bass_kernel_spmd
    nc = _get_program()
    in_maps = _make_in_maps(**inputs)
    res = run_bass_kernel_spmd(nc, in_maps, list(range(N_CORES)), trace=trace)
    # out[b, ep, kt] -> weighted[b, kt*128 + ep]
    outp = np.concatenate(
        [res.results[i]["out"].transpose(0, 2, 1).reshape(BL, O)
         for i in range(N_CORES)], axis=0)
    return outp, res


def kernel(**inputs) -> np.ndarray:
    outp, _ = run(trace=False, **inputs)
    return outp


# revision 12
# speedup vs baseline: 1.2046x; 1.0913x over previous
"""Bass/Trainium2 kernel for nn_Attention (additive attention + weighted sum).

Computation (reference):
    enc  = encoder_outputs.transpose(1, 0, 2)              # [B, S, E]
    z    = enc @ w_e.T + hidden @ w_h.T + attn_b           # [B, S, O]
    att  = softmax(tanh(z) @ v, axis=S)                    # [B, S]
    out  = att @ enc                                       # [B, E]

Sharding: data-parallel over batch — 8 cores x 4 batches each.
Host precomputes hidden @ w_h.T + attn_b (0.1% of FLOPs) and ships the
encoder slice in [b, e, s] layout (contraction over e needs e on
partitions for the big matmul).

Per core, per batch b, per 512-wide s-chunk:
  PE:   energy = tanh(w_e @ enc_chunk + bias)  (fp32r matmuls, full rate)
        scores_chunk = v . energy              (M=1 matmuls)
  then a chunk-local softmax (exp at the chunk max) and the chunk's
  attention-weighted sum run on ACT/DVE/GpSimd, reusing the SAME encoder
  tile from SBUF (multiply + free-axis reduce) while the PE streams the
  next chunk. A final recombine rescales the four chunk-local partial
  sums by exp(m_c - M)/den. The encoder is read from HBM exactly once
  and the PE does nothing but the two contractions.
"""

import numpy as np
from contextlib import ExitStack

# Problem shapes (hardcoded; kernel.py must be self-contained).
B = 32
S = 2048
E = 1024  # encoder hidden
O = 1024  # output dim / attention proj dim
N_CORES = 8
BL = B // N_CORES  # batches per core = 4

P = 128    # partitions
F = 512    # matmul moving free dim (one fp32 PSUM bank)
KE = E // P   # 8 contraction tiles over e
MT = O // P   # 8 output-row tiles over p
NCH = S // F  # 4 s-chunks

_PROGRAM = None


def _build_program():
    import concourse.tile as tile
    from concourse import bacc, mybir

    f32 = mybir.dt.float32
    f32r = mybir.dt.float32r
    AF = mybir.ActivationFunctionType
    AX = mybir.AxisListType
    ALU = mybir.AluOpType

    nc = bacc.Bacc("TRN2", target_bir_lowering=False, debug=False,
                   num_devices=N_CORES)

    encT = nc.dram_tensor("encT", [BL, E, S], f32r, kind="ExternalInput").ap()
    weT = nc.dram_tensor("weT", [P, KE, O], f32r, kind="ExternalInput").ap()
    hb = nc.dram_tensor("hb", [P, MT, BL], f32, kind="ExternalInput").ap()
    vm = nc.dram_tensor("vm", [P, MT], f32r, kind="ExternalInput").ap()
    # out[b, ep, kt] = weighted[b, kt*128 + ep]; host transposes back.
    out = nc.dram_tensor("out", [BL, P, KE], f32, kind="ExternalOutput").ap()

    with tile.TileContext(nc) as tc, ExitStack() as ctx:
        consts = ctx.enter_context(tc.tile_pool(name="consts", bufs=1))
        enc_pool = ctx.enter_context(tc.tile_pool(name="enc", bufs=5))
        epool = ctx.enter_context(tc.tile_pool(name="energy", bufs=8))
        spool = ctx.enter_context(tc.tile_pool(name="scores", bufs=3))
        bpool = ctx.enter_context(tc.tile_pool(name="bcast", bufs=3))
        jpool = ctx.enter_context(tc.tile_pool(name="junk", bufs=2))
        acpool = ctx.enter_context(tc.tile_pool(name="acc", bufs=2))
        opool = ctx.enter_context(tc.tile_pool(name="outsb", bufs=2))
        small = ctx.enter_context(tc.tile_pool(name="small", bufs=8))
        pps = ctx.enter_context(tc.tile_pool(name="pps", bufs=8, space="PSUM"))

        def ps_tile():
            return pps.tile([P, F], f32, tag="ps", name="ps")

        weT_sb = consts.tile([P, KE, O], f32r)
        vm_sb = consts.tile([P, MT], f32r)
        hb_sb = consts.tile([P, MT, BL], f32)

        def load_chunk(b, c):
            # one contiguous [P, KE, F] tile per s-chunk: slice k feeds the
            # pass-A matmuls; the whole tile feeds the DVE weighted sum.
            t = enc_pool.tile([P, KE, F], f32r, tag="ech")
            for k in range(KE):
                nc.sync.dma_start(
                    t[:, k, :], encT[b, k * P:(k + 1) * P, c * F:(c + 1) * F])
            return t

        class BState:
            pass

        def b_begin(b):
            st = BState()
            st.nmrow = small.tile([1, NCH], f32, tag="nmrow", name="nmrow")
            st.denrow = small.tile([1, NCH], f32, tag="denrow", name="denrow")
            st.acc = acpool.tile([P, KE, NCH], f32, tag="acc", name="acc")
            return st

        def chunk_compute(b, c, st, echunk):
            # --- PE: energy + scores for this chunk ---
            energies = []
            for m in range(MT):
                ps = ps_tile()
                for k in range(KE):
                    nc.tensor.matmul(
                        ps[:], weT_sb[:, k, m * P:(m + 1) * P],
                        echunk[:, k, :], start=(k == 0), stop=(k == KE - 1))
                energy = epool.tile([P, F], f32r, tag="energy")
                nc.scalar.activation(energy[:], ps[:], AF.Tanh,
                                     bias=hb_sb[:, m, b:b + 1])
                energies.append(energy)
            sps = ps_tile()
            for m in range(MT):
                nc.tensor.matmul(
                    sps[:1, :], vm_sb[:, m:m + 1], energies[m][:],
                    start=(m == 0), stop=(m == MT - 1))
            srow = spool.tile([1, F], f32, tag="srow", name="srow")
            nc.scalar.copy(srow[:], sps[:1, :])

            # --- chunk-local softmax + weighted partial sum (ACT/DVE/GpSimd,
            # overlaps the next chunk's PE work) ---
            nc.vector.reduce_max(st.nmrow[:, c:c + 1], srow[:], axis=AX.X,
                                 negate=True)
            erow = spool.tile([1, F], f32, tag="erow", name="erow")
            nc.scalar.activation(erow[:], srow[:], AF.Exp,
                                 bias=st.nmrow[:, c:c + 1],
                                 accum_out=st.denrow[:, c:c + 1])
            erow_bc = bpool.tile([P, F], f32, tag="erow_bc", name="erow_bc")
            nc.gpsimd.partition_broadcast(erow_bc[:], erow[:])
            prod = jpool.tile([P, KE, F], f32, tag="junk", name="prod")
            nc.vector.tensor_tensor(
                prod[:], echunk[:].bitcast(f32),
                erow_bc[:, None, :].to_broadcast((P, KE, F)), ALU.mult)
            nc.vector.reduce_sum(st.acc[:, :, c], prod[:], axis=AX.X)

        def b_end(b, st):
            # recombine the chunk-local partials: out = sum_c acc_c *
            # exp(m_c - M) / den_total.
            nmM = small.tile([1, 1], f32, tag="nmM", name="nmM")
            nc.vector.tensor_reduce(nmM[:], st.nmrow[:], axis=AX.X,
                                    op=ALU.min)
            wrow = small.tile([1, NCH], f32, tag="wrow", name="wrow")
            nc.scalar.activation(wrow[:], st.nmrow[:], AF.Exp,
                                 bias=nmM[:], scale=-1.0)
            dtmp = small.tile([1, NCH], f32, tag="dtmp", name="dtmp")
            nc.vector.tensor_tensor(dtmp[:], st.denrow[:], wrow[:], ALU.mult)
            den = small.tile([1, 1], f32, tag="den", name="den")
            nc.vector.reduce_sum(den[:], dtmp[:], axis=AX.X)
            rden = small.tile([1, 1], f32, tag="rden", name="rden")
            nc.vector.reciprocal(rden[:], den[:])
            wrow_bc = bpool.tile([P, NCH], f32, tag="wrow_bc", name="wrow_bc")
            nc.gpsimd.partition_broadcast(wrow_bc[:], wrow[:])
            rden_bc = bpool.tile([P, 1], f32, tag="rden_bc", name="rden_bc")
            nc.gpsimd.partition_broadcast(rden_bc[:], rden[:])
            nc.vector.tensor_tensor(
                st.acc[:], st.acc[:],
                wrow_bc[:, None, :].to_broadcast((P, KE, NCH)), ALU.mult)
            accf = acpool.tile([P, KE], f32, tag="accf", name="accf")
            nc.vector.reduce_sum(accf[:], st.acc[:], axis=AX.X)
            osb = opool.tile([P, KE], f32, tag="osb", name="osb")
            nc.scalar.activation(osb[:], accf[:], AF.Copy, scale=rden_bc[:])
            nc.sync.dma_start(out[b], osb[:])

        # Startup: weights ride the GpSimd DMA queue so the Sync queue
        # delivers the first encoder chunk immediately.
        ech0 = enc_pool.tile([P, KE, F], f32r, tag="ech")
        for k in range(KE):
            nc.sync.dma_start(
                ech0[:, k, :], encT[0, k * P:(k + 1) * P, 0:F])
            if k == 0:
                for m in range(MT):
                    nc.gpsimd.dma_start(weT_sb[:, 0, m * P:(m + 1) * P],
                                        weT[:, 0, m * P:(m + 1) * P])
            else:
                nc.gpsimd.dma_start(weT_sb[:, k, :], weT[:, k, :])
        nc.gpsimd.dma_start(vm_sb[:], vm[:])
        nc.gpsimd.dma_start(hb_sb[:], hb[:])

        # First chunk k-blocked over 8 psum banks: the first matmuls only
        # need weT[k0]+ech0[k0] instead of the full weight prefetch.
        st0 = b_begin(0)
        pstiles = [ps_tile() for _ in range(MT)]
        for k in range(KE):
            for m in range(MT):
                nc.tensor.matmul(
                    pstiles[m][:], weT_sb[:, k, m * P:(m + 1) * P],
                    ech0[:, k, :], start=(k == 0), stop=(k == KE - 1))
        energies = []
        for m in range(MT):
            energy = epool.tile([P, F], f32r, tag="energy")
            nc.scalar.activation(energy[:], pstiles[m][:], AF.Tanh,
                                 bias=hb_sb[:, m, 0:1])
            energies.append(energy)
        sps = ps_tile()
        for m in range(MT):
            nc.tensor.matmul(sps[:1, :], vm_sb[:, m:m + 1], energies[m][:],
                             start=(m == 0), stop=(m == MT - 1))
        srow = spool.tile([1, F], f32, tag="srow", name="srow")
        nc.scalar.copy(srow[:], sps[:1, :])
        nc.vector.reduce_max(st0.nmrow[:, 0:1], srow[:], axis=AX.X,
                             negate=True)
        erow = spool.tile([1, F], f32, tag="erow", name="erow")
        nc.scalar.activation(erow[:], srow[:], AF.Exp, bias=st0.nmrow[:, 0:1],
                             accum_out=st0.denrow[:, 0:1])
        erow_bc = bpool.tile([P, F], f32, tag="erow_bc", name="erow_bc")
        nc.gpsimd.partition_broadcast(erow_bc[:], erow[:])
        prod = jpool.tile([P, KE, F], f32, tag="junk", name="prod")
        nc.vector.tensor_tensor(
            prod[:], ech0[:].bitcast(f32),
            erow_bc[:, None, :].to_broadcast((P, KE, F)),
            mybir.AluOpType.mult)
        nc.vector.reduce_sum(st0.acc[:, :, 0], prod[:], axis=AX.X)

        states = {0: st0}
        for c in range(1, NCH):
            chunk_compute(0, c, st0, load_chunk(0, c))
        for b in range(1, BL):
            states[b] = b_begin(b)
            for c in range(NCH):
                chunk_compute(b, c, states[b], load_chunk(b, c))
            b_end(b - 1, states.pop(b - 1))
        b_end(BL - 1, states.pop(BL - 1))

    nc.compile()
    return nc


def _get_program():
    global _PROGRAM
    if _PROGRAM is None:
        _PROGRAM = _build_program()
    return _PROGRAM


def _make_in_maps(hidden, encoder_outputs, attn_w, attn_b, v):
    hidden = np.asarray(hidden, dtype=np.float32)
    enc = np.asarray(encoder_outputs, dtype=np.float32)
    attn_w = np.asarray(attn_w, dtype=np.float32)
    attn_b = np.asarray(attn_b, dtype=np.float32)
    v = np.asarray(v, dtype=np.float32)

    hb_full = hidden @ attn_w[:, :O].T + attn_b          # [B, O]
    weT = np.ascontiguousarray(
        attn_w[:, O:].T.reshape(KE, P, O).transpose(1, 0, 2))  # [P, KE, O]
    vm = np.ascontiguousarray(v.reshape(MT, P).T)        # [P, MT]

    in_maps = []
    for core in range(N_CORES):
        sl = slice(core * BL, (core + 1) * BL)
        encT_c = np.ascontiguousarray(
            enc[:, sl, :].transpose(1, 2, 0))            # [BL, E, S]
        hb_c = np.ascontiguousarray(
            hb_full[sl].T.reshape(MT, P, BL).transpose(1, 0, 2))  # [P, MT, BL]
        in_maps.append({
            "encT": encT_c,
            "weT": weT,
            "hb": hb_c,
            "vm": vm,
        })
    return in_maps


def run(trace=False, **inputs):
    from concourse.bass_utils import run_bass_kernel_spmd
    nc = _get_program()
    in_maps = _make_in_maps(**inputs)
    res = run_bass_kernel_spmd(nc, in_maps, list(range(N_CORES)), trace=trace)
    # out[b, ep, kt] -> weighted[b, kt*128 + ep]
    outp = np.concatenate(
        [res.results[i]["out"].transpose(0, 2, 1).reshape(BL, O)
         for i in range(N_CORES)], axis=0)
    return outp, res


def kernel(**inputs) -> np.ndarray:
    outp, _ = run(trace=False, **inputs)
    return outp
